# revision 1
# baseline (speedup 1.0000x reference)
"""Single-head attention (B=4, S=4096, D=1024, H=64) on 8 Trainium2 NeuronCores.

kernel(**inputs) takes the FULL unsharded inputs (x, Wq, Wk, Wv as numpy
float32) and returns the FULL output [B, S, H] float32.

Sharding: core c -> batch b = c//2, query-half qh = c%2 (2048 queries each).
The host stages x[b].T as two [1024, 2048] column-halves per core (own half
first, so the device program is identical on every core); each core computes
K/V projections for the full sequence (attention is permutation-invariant in
k, so the "own-half-first" k-order is harmless).

Device pipeline per core (all matmuls in f32r -- TF32-grade, full PE rate):
  proj: xT supertile [128d, 8, 512s] -> K^T_dup [128, 512] (rows 0-63 = K^T,
        rows 64-127 duplicate, built with [Wk|Wk] weights), Q^T_dup likewise,
        V^T [64, 512]; V^T is PE-transposed into V|1 tiles [128, 65] (a ones
        column is appended to V).
  attn: scores are computed TRANSPOSED (k on partitions, q on free) per
        (q-block 512) x (k-chunk pair): two row-group-packed score matmuls
        (contraction h=64 at array rows 0-63 / 64-127) -> PSUM [128, 1024],
        one ScalarE exp with the 1/sqrt(H) fold (exp(0.125*s), no max
        subtraction -- scores are ~N(0,1) so no overflow) -> SBUF attn^T,
        then two matmuls accumulate out^T[65, 512] += [V|1]^T @ attn^T; row
        64 accumulates the softmax denominator.
  epi:  PE-transpose out^T -> [128q, 65], reciprocal of column 64,
        tensor_scalar multiply -> out tile [128, 64] -> DRAM (partition-major
        layout [128, 16, 64]; the host transposes back).
"""

import numpy as np

import concourse.bass as bass
import concourse.mybir as mybir
import concourse.tile as tile
from concourse.bass_utils import run_bass_kernel_spmd
from concourse.masks import make_identity

f32 = mybir.dt.float32
f32r = mybir.dt.float32r
AF = mybir.ActivationFunctionType

B, S, D, H = 4, 4096, 1024, 64
SH = S // 2          # queries per core
NQB = SH // 512      # q-blocks per core
NKC = S // 128       # k-chunks
DC = D // 128        # contraction chunks


def _split_multi_waits(nc):
    """This walrus build rejects >1 sync wait per instruction; split extras
    into preceding NOPs on the same engine."""
    for f in nc.m.functions:
        for bb in f.blocks:
            new_insts = []
            for inst in bb.instructions:
                si = inst.sync_info
                if si is not None and si.on_wait and len(si.on_wait) > 1:
                    waits = list(si.on_wait)
                    for w in waits[:-1]:
                        nop = mybir.InstNoOp(
                            name=nc.get_next_instruction_name(),
                            engine=inst.engine,
                            bass_nofuse=True,
                            sync_info=mybir.SyncInfo(on_wait=[w], on_update=[]),
                        )
                        new_insts.append(nop)
                    inst.sync_info = mybir.SyncInfo(
                        on_wait=[waits[-1]], on_update=list(si.on_update or []))
                new_insts.append(inst)
            bb.instructions = new_insts


def _build_attention_nc():
    nc = bass.Bass("TRN2", target_bir_lowering=False)
    xa = nc.dram_tensor("xa", [D, SH], f32r, kind="ExternalInput")
    xb = nc.dram_tensor("xb", [D, SH], f32r, kind="ExternalInput")
    wqd = nc.dram_tensor("wqd", [D, 128], f32r, kind="ExternalInput")  # [Wq|Wq]
    wkd = nc.dram_tensor("wkd", [D, 128], f32r, kind="ExternalInput")  # [Wk|Wk]
    wv = nc.dram_tensor("wv", [D, H], f32r, kind="ExternalInput")
    out = nc.dram_tensor("out", [128, SH // 128, H], f32, kind="ExternalOutput")

    xa_r = xa.rearrange("(c p) s -> p c s", p=128)   # [128, 8, 2048]
    xb_r = xb.rearrange("(c p) s -> p c s", p=128)

    with tile.TileContext(nc) as tc:
        with (
            tc.tile_pool(name="persist", bufs=1) as persist,
            tc.tile_pool(name="stage", bufs=2) as stage,
            tc.tile_pool(name="attn_sb", bufs=3) as attn_sb,
            tc.tile_pool(name="epi_sb", bufs=2) as epi_sb,
            tc.tile_pool(name="pp", bufs=2, space="PSUM") as pp,
            tc.tile_pool(name="psmall", bufs=1, space="PSUM") as psmall,
            tc.tile_pool(name="psc", bufs=2, space="PSUM") as psc,
            tc.tile_pool(name="pout", bufs=1, space="PSUM") as pout,
        ):
            # ---- constants ----
            wq_sb = persist.tile([128, DC, 128], f32r, tag="wq")
            wk_sb = persist.tile([128, DC, 128], f32r, tag="wk")
            wv_sb = persist.tile([128, DC, H], f32r, tag="wv")
            nc.sync.dma_start(out=wq_sb, in_=wqd.rearrange("(c p) m -> p c m", p=128))
            nc.sync.dma_start(out=wk_sb, in_=wkd.rearrange("(c p) m -> p c m", p=128))
            nc.sync.dma_start(out=wv_sb, in_=wv.rearrange("(c p) m -> p c m", p=128))
            ident = persist.tile([128, 128], f32, tag="ident")
            make_identity(nc, ident)
            ones_sb = persist.tile([128, 1], f32, tag="ones")
            nc.vector.memset(ones_sb, 1.0)

            KT = [persist.tile([128, 512], f32r, tag=f"kt{i}", name=f"KT{i}")
                  for i in range(8)]
            QT = [persist.tile([128, 512], f32r, tag=f"qt{i}", name=f"QT{i}")
                  for i in range(NQB)]
            VO = [persist.tile([128, 4, 65], f32r, tag=f"vo{i}", name=f"VO{i}")
                  for i in range(8)]

            # ---- projection phase ----
            for st in range(8):
                src = xa_r if st < 4 else xb_r
                s0 = (st % 4) * 512
                xt = stage.tile([128, DC, 512], f32r, tag="xt")
                if st == 0:
                    # fine-grained first load so PE starts ASAP
                    for d in range(DC):
                        nc.sync.dma_start(out=xt[:, d, :], in_=src[:, d, s0:s0 + 512])
                else:
                    nc.sync.dma_start(out=xt[:, 0:DC // 2, :],
                                      in_=src[:, 0:DC // 2, s0:s0 + 512])
                    nc.sync.dma_start(out=xt[:, DC // 2:DC, :],
                                      in_=src[:, DC // 2:DC, s0:s0 + 512])

                kt_ps = pp.tile([128, 512], f32, tag="proj_ps", name="kt_ps")
                for d in range(DC):
                    nc.tensor.matmul(kt_ps, wk_sb[:, d, :], xt[:, d, :],
                                     start=(d == 0), stop=(d == DC - 1))
                nc.vector.tensor_copy(out=KT[st], in_=kt_ps)

                vt_ps = pp.tile([64, 512], f32, tag="proj_ps", name="vt_ps")
                for d in range(DC):
                    nc.tensor.matmul(vt_ps, wv_sb[:, d, :], xt[:, d, :],
                                     start=(d == 0), stop=(d == DC - 1))
                vt_sb = stage.tile([64, 512], f32, tag="vt_sb")
                nc.vector.tensor_copy(out=vt_sb, in_=vt_ps)

                if st < 4:  # own half: also project Q
                    qt_ps = pp.tile([128, 512], f32, tag="proj_ps", name="qt_ps")
                    for d in range(DC):
                        nc.tensor.matmul(qt_ps, wq_sb[:, d, :], xt[:, d, :],
                                         start=(d == 0), stop=(d == DC - 1))
                    nc.vector.tensor_copy(out=QT[st], in_=qt_ps)

                # V^T [64, 512] -> V|1 tiles [128, 65]
                for j in range(4):
                    vtr = psmall.tile([128, 65], f32, tag="small", name="vtr")
                    nc.tensor.transpose(vtr[:, 0:64], vt_sb[:, j * 128:(j + 1) * 128],
                                        ident[0:64, 0:64])
                    nc.vector.tensor_copy(out=VO[st][:, j, 0:64], in_=vtr[:, 0:64])
                for j in range(4):
                    nc.vector.tensor_copy(out=VO[st][:, j, 64:65], in_=ones_sb)

            # ---- attention phase ----
            for qb in range(NQB):
                o_ps = pout.tile([65, 512], f32, tag="o_ps")
                for kp in range(NKC // 2):
                    c0, c1 = 2 * kp, 2 * kp + 1
                    st0, j0 = c0 // 4, c0 % 4
                    st1, j1 = c1 // 4, c1 % 4
                    sc = psc.tile([128, 1024], f32, tag="sc")
                    nc.tensor.matmul(sc[:, 0:512],
                                     KT[st0][0:64, j0 * 128:(j0 + 1) * 128],
                                     QT[qb][0:64, :], start=True, stop=True)
                    nc.tensor.matmul(sc[:, 512:1024],
                                     KT[st1][64:128, j1 * 128:(j1 + 1) * 128],
                                     QT[qb][64:128, :], start=True, stop=True)
                    at = attn_sb.tile([128, 1024], f32r, tag="at")
                    nc.scalar.activation(out=at, in_=sc, func=AF.Exp, scale=0.125)
                    nc.tensor.matmul(o_ps, VO[st0][:, j0, :], at[:, 0:512],
                                     start=(kp == 0), stop=False,
                                     skip_group_check=True)
                    nc.tensor.matmul(o_ps, VO[st1][:, j1, :], at[:, 512:1024],
                                     start=False, stop=(kp == NKC // 2 - 1),
                                     skip_group_check=True)

                # epilogue: transpose + divide + store
                o_sb = epi_sb.tile([65, 512], f32, tag="o_sb")
                nc.vector.tensor_copy(out=o_sb, in_=o_ps)
                ot = epi_sb.tile([128, 4, 64], f32, tag="ot")
                for j in range(4):
                    ep = psmall.tile([128, 65], f32, tag="small", name="ep")
                    nc.tensor.transpose(ep, o_sb[:, j * 128:(j + 1) * 128],
                                        ident[0:65, 0:65])
                    rec = epi_sb.tile([128, 1], f32, tag="rec")
                    nc.vector.reciprocal(out=rec, in_=ep[:, 64:65])
                    nc.vector.tensor_scalar_mul(out=ot[:, j, :], in0=ep[:, 0:64],
                                                scalar1=rec)
                nc.sync.dma_start(out=out[:, qb * 4:(qb + 1) * 4, :], in_=ot)

    return nc


_NC_CACHE = {}


def kernel(x, Wq, Wk, Wv):
    x = np.ascontiguousarray(np.asarray(x, dtype=np.float32))
    Wq = np.ascontiguousarray(np.asarray(Wq, dtype=np.float32))
    Wk = np.ascontiguousarray(np.asarray(Wk, dtype=np.float32))
    Wv = np.ascontiguousarray(np.asarray(Wv, dtype=np.float32))
    assert x.shape == (B, S, D) and Wq.shape == (D, H)

    wqd = np.ascontiguousarray(np.concatenate([Wq, Wq], axis=1))
    wkd = np.ascontiguousarray(np.concatenate([Wk, Wk], axis=1))

    in_maps = []
    for c in range(8):
        b, qh = c // 2, c % 2
        xt = np.ascontiguousarray(x[b].T)          # [D, S]
        xa = np.ascontiguousarray(xt[:, qh * SH:(qh + 1) * SH])
        xbo = np.ascontiguousarray(xt[:, (1 - qh) * SH:(2 - qh) * SH])
        in_maps.append({"xa": xa, "xb": xbo, "wqd": wqd, "wkd": wkd, "wv": Wv})

    if "nc" not in _NC_CACHE:
        nc = _build_attention_nc()
        _split_multi_waits(nc)
        _NC_CACHE["nc"] = nc
    nc = _NC_CACHE["nc"]

    res = run_bass_kernel_spmd(nc, in_maps, core_ids=list(range(8)))

    out = np.zeros((B, S, H), dtype=np.float32)
    for c in range(8):
        b, qh = c // 2, c % 2
        r = res.results[c]["out"]                   # [128, 16, 64] partition-major
        out[b, qh * SH:(qh + 1) * SH, :] = r.transpose(1, 0, 2).reshape(SH, H)
    return out
